# revision 4
# baseline (speedup 1.0000x reference)
"""Coupled-map-lattice kernel for Trainium2, data-parallel over 8 NeuronCores.

Reference recurrence (per row n, channels c=0..255, 20 steps):
    mapped = r * g * (1 - g)
    local  = circular 5-tap conv of mapped over c
    glob   = mapped @ W_cc
    g'     = (1-beta)*((1-eps)*mapped + eps*0.5*(local+glob)) + beta*drive
    out    = clip(g_20, 1e-4, 1-1e-4)

Folded form used on device (host precomputes A_neg, qc):
    mapped = r*(1/4 - t),  t = (g - 1/2)^2
    g'     = t @ A_neg + bias2,   bias2 = qc + beta*drive   (constant over steps)
where A[c',c] = (1-beta_c)*[(1-eps_c)*I + eps_c*0.5*(B + W_cc)][c',c],
      B the circulant 5-tap matrix, A_neg = -(r (.)rows A), qc = 1/4 * (r @ A).

Shifted state u = g - 1/2 on device; E = bias2 - 1/2 is a per-chunk constant
tile, so per step:  ps = t @ A_neg (PE),  u' = ps + E,  t' = u'^2.
Work unit is a PSUM "pair" tile [128, 2048] (4 banks); per pair a lane:
  lane P: PE adds E via identity matmuls; ACT squares from PSUM
  lane M: DVE computes u' = ps + E into SBUF f16; ACT squares from SBUF
  lane V: DVE computes u' (as M); DVE squares (u'*u') in f16
Last step is all-P with ACT copy psum->f16; host does clip(out+0.5).
Next chunk's drive DMA is prefetched at the top of each chunk body.
"""

import numpy as np

N, C, KTAPS, STEPS = 131072, 256, 5, 20
N_CORES = 8
N_SHARD = N // N_CORES          # 16384 rows per core
CHUNK = 4096                    # rows resident on-chip per chunk
PSUM_W = 512                    # matmul moving free dim / psum bank width
PAIR_W = 2048                   # psum pair tile width (4 banks)
# lane pattern over pair-slot index (step*4 + j*2 + h); len coprime with 4
LANES = "PMMMMPMMMVPMMMM"

_CACHED_NC = None


def _build_nc():
    import concourse.tile as tile
    from concourse import bacc, mybir

    f32 = mybir.dt.float32
    f16 = mybir.dt.float16
    Act = mybir.ActivationFunctionType
    Alu = mybir.AluOpType

    nc = bacc.Bacc("TRN2", target_bir_lowering=False)
    driveT = nc.declare_dram_parameter("driveT", [C, N_SHARD], f16, isOutput=False)
    a_blk = nc.declare_dram_parameter("a_blk", [128, 640], f32, isOutput=False)
    vecs = nc.declare_dram_parameter("vecs", [128, 4], f32, isOutput=False)
    outT = nc.declare_dram_parameter("outT", [C, N_SHARD], f16, isOutput=True)

    n_chunks = N_SHARD // CHUNK
    n_pairs = CHUNK // PAIR_W            # 2
    n_sub = PAIR_W // PSUM_W             # 4
    n_pt = CHUNK // 1024                 # 1024-granular setup tiles

    with tile.TileContext(nc) as tc:
        with (
            tc.tile_pool(name="const", bufs=1) as constp,
            tc.tile_pool(name="io", bufs=2) as iop,
            tc.tile_pool(name="state", bufs=2) as statep,
            tc.tile_pool(name="u", bufs=2) as up,
            tc.tile_pool(name="psum", bufs=2, space="PSUM") as psump,
        ):
            # ---- constants: A blocks (cols 0-511) + I (cols 512-639), fp16 ----
            a_raw = constp.tile([128, 640], f32)
            nc.gpsimd.dma_start(a_raw[:], a_blk[:])
            a_t = constp.tile([128, 640], f16)
            nc.scalar.copy(a_t[:], a_raw[:])
            v = constp.tile([128, 4], f32)
            nc.gpsimd.dma_start(v[:], vecs[:])
            negh = constp.tile([128, 1], f32)
            nc.vector.memset(negh[:], -0.5)

            def dma_in(ci):
                dd = [iop.tile([128, CHUNK], f16, tag=f"d{j}", name=f"d{j}_{ci}")
                      for j in range(2)]
                col0 = ci * CHUNK
                for j in range(2):
                    nc.gpsimd.dma_start(
                        dd[j][:], driveT[j * 128:(j + 1) * 128, col0:col0 + CHUNK]
                    )
                return dd

            d = dma_in(0)
            for ci in range(n_chunks):
                col0 = ci * CHUNK
                d_next = dma_in(ci + 1) if ci + 1 < n_chunks else None

                tA = [statep.tile([128, CHUNK], f16, tag=f"tA{j}", name=f"tA{j}_{ci}")
                      for j in range(2)]
                tB = [statep.tile([128, CHUNK], f16, tag=f"tB{j}", name=f"tB{j}_{ci}")
                      for j in range(2)]
                # E = beta*drive + qc - 1/2 (constant over steps), f16
                E = [statep.tile([128, CHUNK], f16, tag=f"E{j}", name=f"E{j}_{ci}")
                     for j in range(2)]
                for j in range(2):
                    nc.vector.tensor_scalar(
                        E[j][:], d[j][:], v[:, j:j + 1], v[:, 2 + j:3 + j],
                        Alu.mult, Alu.add,
                    )
                # t0 = (drive - 1/2)^2, 1024-col granular so step-0 matmuls
                # can start early; j=0 on ACT, j=1 on DVE (via u-pool scratch)
                for p in range(n_pt):
                    sl = slice(p * 1024, (p + 1) * 1024)
                    nc.scalar.activation(tA[0][:, sl], d[0][:, sl], Act.Square,
                                         bias=negh[:], scale=1.0)
                for p in range(0, n_pt, 2):
                    sl = slice(p * 1024, (p + 2) * 1024)
                    u0 = up.tile([128, PAIR_W], f16, tag=f"u{p // 2}",
                                 name=f"u0_{ci}_{p}")
                    nc.vector.tensor_scalar_add(u0[:], d[1][:, sl], -0.5)
                    nc.vector.tensor_tensor(tA[1][:, sl], u0[:], u0[:], Alu.mult)

                cur, nxt = tA, tB
                ob = None
                for step in range(STEPS):
                    last = step == STEPS - 1
                    if last:
                        ob = [iop.tile([128, CHUNK], f16, tag=f"ob{j}",
                                       name=f"ob{j}_{ci}") for j in range(2)]
                    for j in range(2):
                        for h in range(n_pairs):
                            ln = "P" if last else \
                                LANES[(step * 4 + j * 2 + h) % len(LANES)]
                            pc0 = h * PAIR_W
                            sl_t = slice(pc0, pc0 + PAIR_W)
                            ps = psump.tile([128, PAIR_W], f32, tag="ps",
                                            name=f"ps_{ci}_{step}_{j}_{h}")
                            for k in range(2):
                                wsl = slice((2 * k + j) * 128,
                                            (2 * k + j + 1) * 128)
                                for s in range(n_sub):
                                    sl_p = slice(s * PSUM_W, (s + 1) * PSUM_W)
                                    c0 = pc0 + s * PSUM_W
                                    nc.tensor.matmul(
                                        ps[:, sl_p], a_t[:, wsl],
                                        cur[k][:, c0:c0 + PSUM_W],
                                        start=k == 0,
                                        stop=k == 1 and ln != "P",
                                    )
                            if ln == "P":
                                for s in range(n_sub):
                                    sl_p = slice(s * PSUM_W, (s + 1) * PSUM_W)
                                    c0 = pc0 + s * PSUM_W
                                    nc.tensor.matmul(
                                        ps[:, sl_p], a_t[:, 512:640],
                                        E[j][:, c0:c0 + PSUM_W],
                                        start=False, stop=True,
                                    )
                                # psum holds u' = t@A + E
                                if last:
                                    nc.scalar.copy(ob[j][:, sl_t], ps[:])
                                else:
                                    nc.scalar.activation(
                                        nxt[j][:, sl_t], ps[:], Act.Square,
                                        bias=0.0, scale=1.0,
                                    )
                            else:
                                u16 = up.tile([128, PAIR_W], f16,
                                              tag=f"u{j * 2 + h}",
                                              name=f"u{j}{h}_{ci}_{step}")
                                nc.vector.tensor_tensor(
                                    u16[:], ps[:], E[j][:, sl_t], Alu.add
                                )
                                if ln == "M":
                                    nc.scalar.activation(
                                        nxt[j][:, sl_t], u16[:], Act.Square,
                                        bias=0.0, scale=1.0,
                                    )
                                else:  # lane V
                                    nc.vector.tensor_tensor(
                                        nxt[j][:, sl_t], u16[:], u16[:], Alu.mult
                                    )
                    cur, nxt = nxt, cur

                for j in range(2):
                    nc.gpsimd.dma_start(
                        outT[j * 128:(j + 1) * 128, col0:col0 + CHUNK], ob[j][:]
                    )
                d = d_next
    nc.compile()
    return nc


def _get_nc():
    global _CACHED_NC
    if _CACHED_NC is None:
        _CACHED_NC = _build_nc()
    return _CACHED_NC


def _fold_constants(r, eps, beta, K_local, W_cc):
    """Host-side fold of the per-step linear operator into A_neg / qc."""
    pad = KTAPS // 2
    cp = np.arange(C)[:, None]
    c = np.arange(C)[None, :]
    j = (cp - c + pad) % C
    B = np.where(j < KTAPS, K_local.astype(np.float64)[np.minimum(j, KTAPS - 1)], 0.0)
    A = (1.0 - beta.astype(np.float64))[None, :] * (
        (1.0 - eps.astype(np.float64))[None, :] * np.eye(C)
        + eps.astype(np.float64)[None, :] * 0.5 * (B + W_cc.astype(np.float64))
    )
    A_r = r.astype(np.float64)[:, None] * A
    A_neg = (-A_r).astype(np.float32)          # [C, C]; u' = t @ A_neg + E
    qc = (0.25 * A_r.sum(axis=0)).astype(np.float32)   # [C]
    return A_neg, qc


def _pack_inputs(drive, r, eps, beta, K_local, W_cc):
    A_neg, qc = _fold_constants(r, eps, beta, K_local, W_cc)
    # lhsT blocks laid out [k0m0 | k0m1 | k1m0 | k1m1 | I]:
    # matmul for output tile m uses cols m*128 (k=0) and (2+m)*128 (k=1)
    blocks = [A_neg[k * 128:(k + 1) * 128, m * 128:(m + 1) * 128]
              for k in range(2) for m in range(2)]
    blocks.append(np.eye(128, dtype=np.float32))
    a_blk = np.concatenate(blocks, axis=1).astype(np.float32)   # [128, 640]
    qcs = qc - np.float32(0.5)
    vecs = np.stack(
        [beta[0:128], beta[128:256], qcs[0:128], qcs[128:256]],
        axis=1,
    ).astype(np.float32)                       # [128, 4]
    driveT = np.ascontiguousarray(drive.T.astype(np.float16))   # [C, N]
    in_maps = []
    for i in range(N_CORES):
        shard = np.ascontiguousarray(driveT[:, i * N_SHARD:(i + 1) * N_SHARD])
        in_maps.append({"driveT": shard, "a_blk": a_blk, "vecs": vecs})
    return in_maps


def run(drive, r, eps, beta, K_local, W_cc, trace=False, trace_kwargs=None):
    from concourse.bass_utils import run_bass_kernel_spmd

    nc = _get_nc()
    in_maps = _pack_inputs(drive, r, eps, beta, K_local, W_cc)
    res = run_bass_kernel_spmd(
        nc, in_maps, core_ids=list(range(N_CORES)),
        trace=trace, **(trace_kwargs or {}),
    )
    outT = np.concatenate([res.results[i]["outT"] for i in range(N_CORES)], axis=1)
    out = np.clip(outT.T.astype(np.float32) + np.float32(0.5),
                  1e-4, 1.0 - 1e-4)
    return np.ascontiguousarray(out), res


def kernel(drive, r, eps, beta, K_local, W_cc):
    out, _ = run(
        np.asarray(drive), np.asarray(r), np.asarray(eps), np.asarray(beta),
        np.asarray(K_local), np.asarray(W_cc),
    )
    return out


# revision 7
# speedup vs baseline: 1.0676x; 1.0676x over previous
"""Coupled-map-lattice kernel for Trainium2, data-parallel over 8 NeuronCores.

Reference recurrence (per row n, channels c=0..255, 20 steps):
    mapped = r * g * (1 - g)
    local  = circular 5-tap conv of mapped over c
    glob   = mapped @ W_cc
    g'     = (1-beta)*((1-eps)*mapped + eps*0.5*(local+glob)) + beta*drive
    out    = clip(g_20, 1e-4, 1-1e-4)

Folded form used on device (host precomputes A_neg, qc):
    mapped = r*(1/4 - t),  t = (g - 1/2)^2
    g'     = t @ A_neg + bias2,   bias2 = qc + beta*drive   (constant over steps)
where A[c',c] = (1-beta_c)*[(1-eps_c)*I + eps_c*0.5*(B + W_cc)][c',c],
      B the circulant 5-tap matrix, A_neg = -(r (.)rows A), qc = 1/4 * (r @ A).

Shifted state u = g - 1/2 on device; E = bias2 - 1/2 is a per-chunk constant
tile, so per step:  ps = t @ A_neg (PE),  u' = ps + E,  t' = u'^2.
Work unit is a PSUM "pair" tile [128, 2048] (4 banks); per pair a lane:
  lane P: PE adds E via identity matmuls; ACT squares from PSUM
  lane M: DVE computes u' = ps + E into SBUF f16; ACT squares from SBUF
  lane V: DVE computes u' (as M); DVE squares (u'*u') in f16
Last step is all-P with ACT copy psum->f16; host does clip(out+0.5).
Next chunk's drive DMA is prefetched at the top of each chunk body.
"""

import numpy as np

N, C, KTAPS, STEPS = 131072, 256, 5, 20
N_CORES = 8
N_SHARD = N // N_CORES          # 16384 rows per core
CHUNK = 4096                    # rows resident on-chip per chunk
PSUM_W = 512                    # matmul moving free dim / psum bank width
PAIR_W = 2048                   # psum pair tile width (4 banks)
# lane pattern over pair-slot index (step*4 + j*2 + h); len coprime with 4
LANES = "PMMVPMMMPMMMPMM"

_CACHED_NC = None


def _build_nc():
    import concourse.tile as tile
    from concourse import bacc, mybir

    f32 = mybir.dt.float32
    f16 = mybir.dt.float16
    Act = mybir.ActivationFunctionType
    Alu = mybir.AluOpType

    nc = bacc.Bacc("TRN2", target_bir_lowering=False)
    driveT = nc.declare_dram_parameter("driveT", [C, N_SHARD], f16, isOutput=False)
    a_blk = nc.declare_dram_parameter("a_blk", [128, 640], f32, isOutput=False)
    vecs = nc.declare_dram_parameter("vecs", [128, 4], f32, isOutput=False)
    outT = nc.declare_dram_parameter("outT", [C, N_SHARD], f16, isOutput=True)

    n_chunks = N_SHARD // CHUNK
    n_pairs = CHUNK // PAIR_W            # 2
    n_sub = PAIR_W // PSUM_W             # 4
    n_pt = CHUNK // 1024                 # 1024-granular setup tiles

    with tile.TileContext(nc) as tc:
        with (
            tc.tile_pool(name="const", bufs=1) as constp,
            tc.tile_pool(name="io", bufs=2) as iop,
            tc.tile_pool(name="state", bufs=2) as statep,
            tc.tile_pool(name="u", bufs=2) as up,
            tc.tile_pool(name="psum", bufs=4, space="PSUM") as psump,
        ):
            # ---- constants: A blocks (cols 0-511) + I (cols 512-639), fp16 ----
            a_raw = constp.tile([128, 640], f32)
            nc.gpsimd.dma_start(a_raw[:], a_blk[:])
            a_t = constp.tile([128, 640], f16)
            nc.scalar.copy(a_t[:], a_raw[:])
            v = constp.tile([128, 4], f32)
            nc.gpsimd.dma_start(v[:], vecs[:])
            negh = constp.tile([128, 1], f32)
            nc.vector.memset(negh[:], -0.5)

            def dma_in(ci):
                dd = [iop.tile([128, CHUNK], f16, tag=f"d{j}", name=f"d{j}_{ci}")
                      for j in range(2)]
                col0 = ci * CHUNK
                for j in range(2):
                    nc.gpsimd.dma_start(
                        dd[j][:], driveT[j * 128:(j + 1) * 128, col0:col0 + CHUNK]
                    )
                return dd

            d = dma_in(0)
            for ci in range(n_chunks):
                col0 = ci * CHUNK
                d_next = dma_in(ci + 1) if ci + 1 < n_chunks else None

                tA = [statep.tile([128, CHUNK], f16, tag=f"tA{j}", name=f"tA{j}_{ci}")
                      for j in range(2)]
                tB = [statep.tile([128, CHUNK], f16, tag=f"tB{j}", name=f"tB{j}_{ci}")
                      for j in range(2)]
                # E = beta*drive + qc - 1/2 (constant over steps), f16
                E = [statep.tile([128, CHUNK], f16, tag=f"E{j}", name=f"E{j}_{ci}")
                     for j in range(2)]
                for j in range(2):
                    nc.vector.tensor_scalar(
                        E[j][:], d[j][:], v[:, j:j + 1], v[:, 2 + j:3 + j],
                        Alu.mult, Alu.add,
                    )
                # t0 = (drive - 1/2)^2, 1024-col granular so step-0 matmuls
                # can start early; j=0 on ACT, j=1 on DVE (via u-pool scratch)
                for p in range(n_pt):
                    sl = slice(p * 1024, (p + 1) * 1024)
                    nc.scalar.activation(tA[0][:, sl], d[0][:, sl], Act.Square,
                                         bias=negh[:], scale=1.0)
                for p in range(0, n_pt, 2):
                    sl = slice(p * 1024, (p + 2) * 1024)
                    u0 = up.tile([128, PAIR_W], f16, tag=f"u{p // 2}",
                                 name=f"u0_{ci}_{p}")
                    nc.vector.tensor_scalar_add(u0[:], d[1][:, sl], -0.5)
                    nc.vector.tensor_tensor(tA[1][:, sl], u0[:], u0[:], Alu.mult)

                cur, nxt = tA, tB
                ob = None
                for step in range(STEPS):
                    last = step == STEPS - 1
                    if last:
                        ob = [iop.tile([128, CHUNK], f16, tag=f"ob{j}",
                                       name=f"ob{j}_{ci}") for j in range(2)]
                    for j in range(2):
                        for h in range(n_pairs):
                            ln = "P" if last else \
                                LANES[(step * 4 + j * 2 + h) % len(LANES)]
                            pc0 = h * PAIR_W
                            sl_t = slice(pc0, pc0 + PAIR_W)
                            u16 = None
                            if ln != "P":
                                u16 = up.tile([128, PAIR_W], f16,
                                              tag=f"u{j * 2 + h}",
                                              name=f"u{j}{h}_{ci}_{step}")
                            # two 1024-wide psum tiles (2 banks each) per pair
                            for q in range(2):
                                qc0 = pc0 + q * 1024
                                ps = psump.tile([128, 1024], f32, tag="ps",
                                                name=f"ps_{ci}_{step}_{j}_{h}{q}")
                                for s in range(2):
                                    sl_p = slice(s * PSUM_W, (s + 1) * PSUM_W)
                                    c0 = qc0 + s * PSUM_W
                                    sl_c = slice(c0, c0 + PSUM_W)
                                    nc.tensor.matmul(
                                        ps[:, sl_p], a_t[:, j * 128:(j + 1) * 128],
                                        cur[0][:, sl_c], start=True, stop=False,
                                    )
                                    nc.tensor.matmul(
                                        ps[:, sl_p],
                                        a_t[:, (2 + j) * 128:(3 + j) * 128],
                                        cur[1][:, sl_c], start=False,
                                        stop=ln != "P",
                                    )
                                    if ln == "P":
                                        nc.tensor.matmul(
                                            ps[:, sl_p], a_t[:, 512:640],
                                            E[j][:, sl_c], start=False, stop=True,
                                        )
                                sl_q = slice(qc0, qc0 + 1024)
                                if ln == "P":
                                    # psum holds u' = t@A + E
                                    if last:
                                        nc.scalar.copy(ob[j][:, sl_q], ps[:])
                                    else:
                                        nc.scalar.activation(
                                            nxt[j][:, sl_q], ps[:], Act.Square,
                                            bias=0.0, scale=1.0,
                                        )
                                else:
                                    nc.vector.tensor_tensor(
                                        u16[:, q * 1024:(q + 1) * 1024], ps[:],
                                        E[j][:, sl_q], Alu.add,
                                    )
                            if ln == "M":
                                nc.scalar.activation(
                                    nxt[j][:, sl_t], u16[:], Act.Square,
                                    bias=0.0, scale=1.0,
                                )
                            elif ln == "V":
                                nc.vector.tensor_tensor(
                                    nxt[j][:, sl_t], u16[:], u16[:], Alu.mult
                                )
                    cur, nxt = nxt, cur

                for j in range(2):
                    nc.gpsimd.dma_start(
                        outT[j * 128:(j + 1) * 128, col0:col0 + CHUNK], ob[j][:]
                    )
                d = d_next
    nc.compile()
    return nc


def _get_nc():
    global _CACHED_NC
    if _CACHED_NC is None:
        _CACHED_NC = _build_nc()
    return _CACHED_NC


def _fold_constants(r, eps, beta, K_local, W_cc):
    """Host-side fold of the per-step linear operator into A_neg / qc."""
    pad = KTAPS // 2
    cp = np.arange(C)[:, None]
    c = np.arange(C)[None, :]
    j = (cp - c + pad) % C
    B = np.where(j < KTAPS, K_local.astype(np.float64)[np.minimum(j, KTAPS - 1)], 0.0)
    A = (1.0 - beta.astype(np.float64))[None, :] * (
        (1.0 - eps.astype(np.float64))[None, :] * np.eye(C)
        + eps.astype(np.float64)[None, :] * 0.5 * (B + W_cc.astype(np.float64))
    )
    A_r = r.astype(np.float64)[:, None] * A
    A_neg = (-A_r).astype(np.float32)          # [C, C]; u' = t @ A_neg + E
    qc = (0.25 * A_r.sum(axis=0)).astype(np.float32)   # [C]
    return A_neg, qc


def _pack_inputs(drive, r, eps, beta, K_local, W_cc):
    A_neg, qc = _fold_constants(r, eps, beta, K_local, W_cc)
    # lhsT blocks laid out [k0m0 | k0m1 | k1m0 | k1m1 | I]:
    # matmul for output tile m uses cols m*128 (k=0) and (2+m)*128 (k=1)
    blocks = [A_neg[k * 128:(k + 1) * 128, m * 128:(m + 1) * 128]
              for k in range(2) for m in range(2)]
    blocks.append(np.eye(128, dtype=np.float32))
    a_blk = np.concatenate(blocks, axis=1).astype(np.float32)   # [128, 640]
    qcs = qc - np.float32(0.5)
    vecs = np.stack(
        [beta[0:128], beta[128:256], qcs[0:128], qcs[128:256]],
        axis=1,
    ).astype(np.float32)                       # [128, 4]
    driveT = np.ascontiguousarray(drive.T.astype(np.float16))   # [C, N]
    in_maps = []
    for i in range(N_CORES):
        shard = np.ascontiguousarray(driveT[:, i * N_SHARD:(i + 1) * N_SHARD])
        in_maps.append({"driveT": shard, "a_blk": a_blk, "vecs": vecs})
    return in_maps


def run(drive, r, eps, beta, K_local, W_cc, trace=False, trace_kwargs=None):
    from concourse.bass_utils import run_bass_kernel_spmd

    nc = _get_nc()
    in_maps = _pack_inputs(drive, r, eps, beta, K_local, W_cc)
    res = run_bass_kernel_spmd(
        nc, in_maps, core_ids=list(range(N_CORES)),
        trace=trace, **(trace_kwargs or {}),
    )
    outT = np.concatenate([res.results[i]["outT"] for i in range(N_CORES)], axis=1)
    out = np.clip(outT.T.astype(np.float32) + np.float32(0.5),
                  1e-4, 1.0 - 1e-4)
    return np.ascontiguousarray(out), res


def kernel(drive, r, eps, beta, K_local, W_cc):
    out, _ = run(
        np.asarray(drive), np.asarray(r), np.asarray(eps), np.asarray(beta),
        np.asarray(K_local), np.asarray(W_cc),
    )
    return out


# revision 8
# speedup vs baseline: 1.2712x; 1.1907x over previous
"""Coupled-map-lattice kernel for Trainium2, data-parallel over 8 NeuronCores.

Reference recurrence (per row n, channels c=0..255, 20 steps):
    mapped = r * g * (1 - g)
    local  = circular 5-tap conv of mapped over c
    glob   = mapped @ W_cc
    g'     = (1-beta)*((1-eps)*mapped + eps*0.5*(local+glob)) + beta*drive
    out    = clip(g_20, 1e-4, 1-1e-4)

Folded form used on device (host precomputes A_neg, qc):
    mapped = r*(1/4 - t),  t = (g - 1/2)^2
    g'     = t @ A_neg + bias2,   bias2 = qc + beta*drive   (constant over steps)
where A[c',c] = (1-beta_c)*[(1-eps_c)*I + eps_c*0.5*(B + W_cc)][c',c],
      B the circulant 5-tap matrix, A_neg = -(r (.)rows A), qc = 1/4 * (r @ A).

Shifted state u = g - 1/2 on device; E = bias2 - 1/2 is a per-chunk constant
tile, so per step:  ps = t @ A_neg (PE),  u' = ps + E,  t' = u'^2.
Per psum tile [128, 1024] a lane balances the engines:
  lane P: PE adds E via identity matmuls; ACT squares from PSUM
  lane M: DVE computes u' = ps + E into SBUF f16; ACT squares from SBUF
  lane V: DVE computes u' (as M); DVE squares (u'*u') in f16
Last step is all-P; psum evacuated to f16 by ACT copies (even tiles) and DVE
copies (odd tiles); host does clip(out + 0.5).  Next chunk's drive DMA is
prefetched at the top of each chunk body.
"""

import numpy as np

N, C, KTAPS, STEPS = 131072, 256, 5, 20
N_CORES = 8
N_SHARD = N // N_CORES          # 16384 rows per core
CHUNK = 4096                    # rows resident on-chip per chunk
PSUM_W = 512                    # matmul moving free dim / psum bank width
PSUM_TILE_W = 1024              # psum tile width (2 banks)
# lane of tile index (step*8 + j*4 + p); len coprime with 8
LANES = "PMMVPMMMPMVMMPMVMPMM"

_CACHED_NC = None


def _build_nc():
    import concourse.tile as tile
    from concourse import bacc, mybir

    f32 = mybir.dt.float32
    f16 = mybir.dt.float16
    Act = mybir.ActivationFunctionType
    Alu = mybir.AluOpType

    nc = bacc.Bacc("TRN2", target_bir_lowering=False)
    driveT = nc.declare_dram_parameter("driveT", [C, N_SHARD], f16, isOutput=False)
    a_blk = nc.declare_dram_parameter("a_blk", [128, 640], f32, isOutput=False)
    vecs = nc.declare_dram_parameter("vecs", [128, 4], f32, isOutput=False)
    outT = nc.declare_dram_parameter("outT", [C, N_SHARD], f16, isOutput=True)

    n_chunks = N_SHARD // CHUNK
    n_ptiles = CHUNK // PSUM_TILE_W
    n_sub = PSUM_TILE_W // PSUM_W

    with tile.TileContext(nc) as tc:
        with (
            tc.tile_pool(name="const", bufs=1) as constp,
            tc.tile_pool(name="io", bufs=2) as iop,
            tc.tile_pool(name="state", bufs=2) as statep,
            tc.tile_pool(name="u", bufs=2) as up,
            tc.tile_pool(name="psum", bufs=4, space="PSUM") as psump,
        ):
            # ---- constants: A blocks (cols 0-511) + I (cols 512-639), fp16 ----
            a_raw = constp.tile([128, 640], f32)
            nc.gpsimd.dma_start(a_raw[:], a_blk[:])
            a_t = constp.tile([128, 640], f16)
            nc.scalar.copy(a_t[:], a_raw[:])
            v = constp.tile([128, 4], f32)
            nc.gpsimd.dma_start(v[:], vecs[:])
            negh = constp.tile([128, 1], f32)
            nc.vector.memset(negh[:], -0.5)

            def dma_in(ci):
                dd = [iop.tile([128, CHUNK], f16, tag=f"d{j}", name=f"d{j}_{ci}")
                      for j in range(2)]
                col0 = ci * CHUNK
                for j in range(2):
                    nc.gpsimd.dma_start(
                        dd[j][:], driveT[j * 128:(j + 1) * 128, col0:col0 + CHUNK]
                    )
                return dd

            d = dma_in(0)
            for ci in range(n_chunks):
                col0 = ci * CHUNK
                d_next = dma_in(ci + 1) if ci + 1 < n_chunks else None

                tA = [statep.tile([128, CHUNK], f16, tag=f"tA{j}", name=f"tA{j}_{ci}")
                      for j in range(2)]
                tB = [statep.tile([128, CHUNK], f16, tag=f"tB{j}", name=f"tB{j}_{ci}")
                      for j in range(2)]
                # E = beta*drive + qc - 1/2 (constant over steps), f16
                E = [statep.tile([128, CHUNK], f16, tag=f"E{j}", name=f"E{j}_{ci}")
                     for j in range(2)]
                for j in range(2):
                    nc.vector.tensor_scalar(
                        E[j][:], d[j][:], v[:, j:j + 1], v[:, 2 + j:3 + j],
                        Alu.mult, Alu.add,
                    )
                # t0 = (drive - 1/2)^2, 1024-col granular so step-0 matmuls
                # can start early; j=0 on ACT, j=1 on DVE (via u-pool scratch)
                for p in range(n_ptiles):
                    sl = slice(p * PSUM_TILE_W, (p + 1) * PSUM_TILE_W)
                    nc.scalar.activation(tA[0][:, sl], d[0][:, sl], Act.Square,
                                         bias=negh[:], scale=1.0)
                for p in range(n_ptiles):
                    sl = slice(p * PSUM_TILE_W, (p + 1) * PSUM_TILE_W)
                    u0 = up.tile([128, PSUM_TILE_W], f16, tag=f"u{p}",
                                 name=f"u0_{ci}_{p}")
                    nc.vector.tensor_scalar_add(u0[:], d[1][:, sl], -0.5)
                    nc.vector.tensor_tensor(tA[1][:, sl], u0[:], u0[:], Alu.mult)

                cur, nxt = tA, tB
                ob = None
                for step in range(STEPS):
                    last = step == STEPS - 1
                    if last:
                        ob = [iop.tile([128, CHUNK], f16, tag=f"ob{j}",
                                       name=f"ob{j}_{ci}") for j in range(2)]
                    for j in range(2):
                        for p in range(n_ptiles):
                            ln = "P" if last else \
                                LANES[(step * 8 + j * 4 + p) % len(LANES)]
                            pc0 = p * PSUM_TILE_W
                            sl_t = slice(pc0, pc0 + PSUM_TILE_W)
                            ps = psump.tile([128, PSUM_TILE_W], f32, tag="ps",
                                            name=f"ps_{ci}_{step}_{j}_{p}")
                            for s in range(n_sub):
                                sl_p = slice(s * PSUM_W, (s + 1) * PSUM_W)
                                c0 = pc0 + s * PSUM_W
                                sl_c = slice(c0, c0 + PSUM_W)
                                nc.tensor.matmul(
                                    ps[:, sl_p], a_t[:, j * 128:(j + 1) * 128],
                                    cur[0][:, sl_c], start=True, stop=False,
                                )
                                nc.tensor.matmul(
                                    ps[:, sl_p], a_t[:, (2 + j) * 128:(3 + j) * 128],
                                    cur[1][:, sl_c], start=False, stop=ln != "P",
                                )
                                if ln == "P":
                                    nc.tensor.matmul(
                                        ps[:, sl_p], a_t[:, 512:640],
                                        E[j][:, sl_c], start=False, stop=True,
                                    )
                            if ln == "P":
                                # psum holds u' = t@A + E
                                if last:
                                    if p % 2 == 0:
                                        nc.scalar.copy(ob[j][:, sl_t], ps[:])
                                    else:
                                        nc.vector.tensor_scalar_add(
                                            ob[j][:, sl_t], ps[:], 0.0
                                        )
                                else:
                                    nc.scalar.activation(
                                        nxt[j][:, sl_t], ps[:], Act.Square,
                                        bias=0.0, scale=1.0,
                                    )
                            else:
                                u16 = up.tile([128, PSUM_TILE_W], f16,
                                              tag=f"u{j * 2 + (p % 2)}",
                                              name=f"u{j}{p}_{ci}_{step}")
                                nc.vector.tensor_tensor(
                                    u16[:], ps[:], E[j][:, sl_t], Alu.add
                                )
                                if ln == "M":
                                    nc.scalar.activation(
                                        nxt[j][:, sl_t], u16[:], Act.Square,
                                        bias=0.0, scale=1.0,
                                    )
                                else:  # lane V
                                    nc.vector.tensor_tensor(
                                        nxt[j][:, sl_t], u16[:], u16[:], Alu.mult
                                    )
                    cur, nxt = nxt, cur

                for j in range(2):
                    nc.gpsimd.dma_start(
                        outT[j * 128:(j + 1) * 128, col0:col0 + CHUNK], ob[j][:]
                    )
                d = d_next
    nc.compile()
    return nc


def _get_nc():
    global _CACHED_NC
    if _CACHED_NC is None:
        _CACHED_NC = _build_nc()
    return _CACHED_NC


def _fold_constants(r, eps, beta, K_local, W_cc):
    """Host-side fold of the per-step linear operator into A_neg / qc."""
    pad = KTAPS // 2
    cp = np.arange(C)[:, None]
    c = np.arange(C)[None, :]
    j = (cp - c + pad) % C
    B = np.where(j < KTAPS, K_local.astype(np.float64)[np.minimum(j, KTAPS - 1)], 0.0)
    A = (1.0 - beta.astype(np.float64))[None, :] * (
        (1.0 - eps.astype(np.float64))[None, :] * np.eye(C)
        + eps.astype(np.float64)[None, :] * 0.5 * (B + W_cc.astype(np.float64))
    )
    A_r = r.astype(np.float64)[:, None] * A
    A_neg = (-A_r).astype(np.float32)          # [C, C]; u' = t @ A_neg + E
    qc = (0.25 * A_r.sum(axis=0)).astype(np.float32)   # [C]
    return A_neg, qc


def _pack_inputs(drive, r, eps, beta, K_local, W_cc):
    A_neg, qc = _fold_constants(r, eps, beta, K_local, W_cc)
    # lhsT blocks laid out [k0m0 | k0m1 | k1m0 | k1m1 | I]:
    # matmul for output tile m uses cols m*128 (k=0) and (2+m)*128 (k=1)
    blocks = [A_neg[k * 128:(k + 1) * 128, m * 128:(m + 1) * 128]
              for k in range(2) for m in range(2)]
    blocks.append(np.eye(128, dtype=np.float32))
    a_blk = np.concatenate(blocks, axis=1).astype(np.float32)   # [128, 640]
    qcs = qc - np.float32(0.5)
    vecs = np.stack(
        [beta[0:128], beta[128:256], qcs[0:128], qcs[128:256]],
        axis=1,
    ).astype(np.float32)                       # [128, 4]
    driveT = np.ascontiguousarray(drive.T.astype(np.float16))   # [C, N]
    in_maps = []
    for i in range(N_CORES):
        shard = np.ascontiguousarray(driveT[:, i * N_SHARD:(i + 1) * N_SHARD])
        in_maps.append({"driveT": shard, "a_blk": a_blk, "vecs": vecs})
    return in_maps


def run(drive, r, eps, beta, K_local, W_cc, trace=False, trace_kwargs=None):
    from concourse.bass_utils import run_bass_kernel_spmd

    nc = _get_nc()
    in_maps = _pack_inputs(drive, r, eps, beta, K_local, W_cc)
    res = run_bass_kernel_spmd(
        nc, in_maps, core_ids=list(range(N_CORES)),
        trace=trace, **(trace_kwargs or {}),
    )
    outT = np.concatenate([res.results[i]["outT"] for i in range(N_CORES)], axis=1)
    out = np.clip(outT.T.astype(np.float32) + np.float32(0.5),
                  1e-4, 1.0 - 1e-4)
    return np.ascontiguousarray(out), res


def kernel(drive, r, eps, beta, K_local, W_cc):
    out, _ = run(
        np.asarray(drive), np.asarray(r), np.asarray(eps), np.asarray(beta),
        np.asarray(K_local), np.asarray(W_cc),
    )
    return out
